# revision 24
# baseline (speedup 1.0000x reference)
"""Single-head attention (S=8192, D=1024, d_k=128) on 8 TRN2 NeuronCores.

Sequence-parallel: each core owns SL=1024 query rows; K/V are All-Gathered.

Numerics strategy (tolerance rel_err < 2e-2; scores ~ N(0, 1200^2) make the
softmax near-one-hot, so Q/K precision decides argmax flips):
  - Q/K/V projections: explicit hi/lo bf16 split (x = xh + xl, W = Wh + Wl;
    keep xh*Wh + xh*Wl + xl*Wh) -> ~16-bit mantissa quality at 3 bf16-rate
    matmuls instead of one 4x-slower fp32 matmul.
  - Scores pass (exact): fp32r matmuls (fp32 operands rounded to 11 mantissa
    bits, full-rate on the PE when moving free >= 256). Measured on HW:
    identical to 11-bit round-nearest; end-to-end sim rel_err ~6e-3.
  - Max pass: separate bf16 score pass (max only needs +-80 accuracy).
  - P (exp scores), V, PV matmul: bf16 (graceful, renormalized errors).

Schedule per core:
  - transpose x hi/lo via bf16 PE transposes; hi/lo projections
  - K first: round K^T to fp32r bits + bf16 copy; AllGather Kh (bf16),
    then K (fp32r bits), then V natural (bf16); a tiny warmup collective is
    issued first so the ~45us CC barrier overlaps the projection phase.
  - per 512-query block g:
      pass1: S~^T tiles = Kh-slice.T @ Qh (bf16) in [k, q] orientation;
             DVE running max -> macc; transpose-reduce -> -m per q.
      pass2: per k-tile: rank-1 matmul seeds PSUM with -m broadcast
             (1-row ldweights, cheap), fp32r scores accumulate on top,
             ACT exp -> P^T bf16 in SBUF, PE accumulates O^T += V.T @ P^T,
             gpsimd accumulates l over 4-tile batches.
      tail: transpose-reduce l, reciprocal, rank-1 broadcast 1/l,
            normalize, PE-transpose O^T -> O, DMA out.

Toolchain constraint: walrus allows at most ONE sync wait per non-sequencer
instruction; split_multi_waits() hoists extras onto EventSemaphore, and every
DMA-fed matmul operand gets a tiny ldweights "absorber" so real matmuls
never wait on DMA queues directly.
"""

import math
import os
import sys
from contextlib import ExitStack

for _p in ("/opt/trn_rl_repo", os.path.expanduser("~/.axon_site/_ro/trn_rl_repo")):
    if os.path.isdir(_p) and _p not in sys.path:
        sys.path.insert(0, _p)

import numpy as np

import concourse.bass as bass
import concourse.mybir as mybir
import concourse.tile as tile
from concourse.bass_utils import run_bass_kernel_spmd
from concourse.masks import make_identity

S = 8192
D = 1024
DK = 128
NC = 8
SL = S // NC  # 1024 query rows per core
NQ = 512      # queries per block
KB = 4        # k-tiles per gpsimd l batch
SCALE = 1.0 / math.sqrt(DK)
FP32 = mybir.dt.float32
FP32R = mybir.dt.float32r
BF16 = mybir.dt.bfloat16
Act = mybir.ActivationFunctionType
Alu = mybir.AluOpType


def build_program() -> bass.Bass:
    nc = bass.Bass(num_devices=NC)

    x_sh = nc.declare_dram_parameter("x_sh", [SL, D], FP32, isOutput=False)
    w_q = nc.declare_dram_parameter("W_Q", [D, DK], FP32, isOutput=False)
    b_q = nc.declare_dram_parameter("b_Q", [1, DK], FP32, isOutput=False)
    w_k = nc.declare_dram_parameter("W_K", [D, DK], FP32, isOutput=False)
    b_k = nc.declare_dram_parameter("b_K", [1, DK], FP32, isOutput=False)
    w_v = nc.declare_dram_parameter("W_V", [D, DK], FP32, isOutput=False)
    b_v = nc.declare_dram_parameter("b_V", [1, DK], FP32, isOutput=False)
    out_sh = nc.declare_dram_parameter("out_sh", [SL, DK], FP32, isOutput=True)

    groups = [list(range(NC))]

    with tile.TileContext(nc) as tc, ExitStack() as ctx:
        dram = ctx.enter_context(tc.tile_pool(name="dram", bufs=1, space="DRAM"))
        ktl1_d = dram.tile([DK, SL // 2], FP32)  # fp32r bits, first seq half
        ktl2_d = dram.tile([DK, SL // 2], FP32)
        ktg1_d = dram.tile([NC * DK, SL // 2], FP32, addr_space="Shared")
        ktg2_d = dram.tile([NC * DK, SL // 2], FP32, addr_space="Shared")
        vnl_d = dram.tile([SL, DK], BF16)
        vng_d = dram.tile([S, DK], BF16, addr_space="Shared")

        const = ctx.enter_context(tc.tile_pool(name="const", bufs=1))
        big = ctx.enter_context(tc.tile_pool(name="big", bufs=1))
        stat = ctx.enter_context(tc.tile_pool(name="stat", bufs=2))
        rows = ctx.enter_context(tc.tile_pool(name="rows", bufs=2))
        pbufp = ctx.enter_context(tc.tile_pool(name="pbuf", bufs=12))
        outp = ctx.enter_context(tc.tile_pool(name="outp", bufs=3))
        # PSUM budget (8 banks): "ps" fp32 x4 + pass1 "p1" x2 + psO + psL
        psS = ctx.enter_context(tc.tile_pool(name="psS", bufs=4, space="PSUM"))
        psP = ctx.enter_context(tc.tile_pool(name="psP", bufs=2, space="PSUM"))
        psacc = ctx.enter_context(tc.tile_pool(name="psacc", bufs=1, space="PSUM"))

        def absorb(col_ap):
            """1-wait PE ldweights folding col_ap's producer sem into PE's clock."""
            if col_ap.dtype != BF16:
                col_ap = col_ap.bitcast(BF16)
            nc.tensor.ldweights(weights=col_ap)

        ident = const.tile([128, 128], FP32)
        make_identity(nc, ident[:, :])
        absorb(ident[:, 0:1])
        dummy = const.tile([128, 512], BF16, tag="dummy")
        nc.gpsimd.memset(dummy[0:1, 0:1], 1.0)

        def warm_burst(n, lhs=None):
            """Back-to-back throwaway matmuls to raise the HAM clock gate."""
            for i in range(n):
                pw = psS.tile([128, 512], FP32, tag="ps", name=f"warm{i}")
                nc.tensor.matmul(
                    pw[:, :], lhsT=(lhs if lhs is not None else dummy[:, 0:128]),
                    rhs=dummy[:, :], start=True, stop=True, skip_group_check=True,
                )
        warm_burst(16)
        identb = const.tile([128, 128], BF16)
        nc.vector.tensor_copy(identb[:, :], ident[:, :])
        ones_rb = const.tile([1, 128], BF16, tag="ones_rb")
        nc.gpsimd.memset(ones_rb[:, :], 1.0)
        ones_rf = const.tile([1, 128], FP32, tag="ones_rf")
        nc.gpsimd.memset(ones_rf[:, :], 1.0)

        def load_bias_T(b_dram, tag):
            t = const.tile([128, 1], FP32, tag=tag)
            nc.sync.dma_start(out=t[:, 0], in_=b_dram[0, :])
            return t

        bqT = load_bias_T(b_q, "bqT")
        bkT = load_bias_T(b_k, "bkT")
        bvT = load_bias_T(b_v, "bvT")
        bqTs = const.tile([128, 1], FP32, tag="bqTs")
        nc.scalar.mul(bqTs[:, :], bqT[:, :], SCALE)

        # ================= projection phase =================
        proj_ctx = ExitStack()
        xpool = proj_ctx.enter_context(tc.tile_pool(name="xpool", bufs=1))
        xload = proj_ctx.enter_context(tc.tile_pool(name="xload", bufs=2))
        xcvt = proj_ctx.enter_context(tc.tile_pool(name="xcvt", bufs=2))
        wpool = proj_ctx.enter_context(tc.tile_pool(name="wpool", bufs=2))

        # x^T hi/lo: [128 d-part, 8 d-tiles, SL seq] bf16
        xhT = xpool.tile([128, D // 128, SL], BF16, tag="xhT")
        xlT = xpool.tile([128, D // 128, SL], BF16, tag="xlT")
        for st in range(SL // 128):
            xn = xload.tile([128, D], FP32)
            nc.gpsimd.dma_start(out=xn[:, :], in_=x_sh[st * 128 : (st + 1) * 128, :])
            absorb(xn[:, 0:1])
            xhb = xcvt.tile([128, D], BF16, tag="xhb")
            nc.scalar.copy(xhb[:, :], xn[:, :])
            nc.vector.tensor_sub(xn[:, :], xn[:, :], xhb[:, :])
            xlb = xcvt.tile([128, D], BF16, tag="xlb")
            nc.scalar.copy(xlb[:, :], xn[:, :])
            for dt in range(D // 128):
                ph = psS.tile([128, 128], BF16, tag="ps")
                nc.tensor.transpose(ph[:, :], xhb[:, dt * 128 : (dt + 1) * 128], identb[:, :])
                nc.vector.tensor_copy(xhT[:, dt, st * 128 : (st + 1) * 128], ph[:, :])
                pl = psS.tile([128, 128], BF16, tag="ps")
                nc.tensor.transpose(pl[:, :], xlb[:, dt * 128 : (dt + 1) * 128], identb[:, :])
                nc.vector.tensor_copy(xlT[:, dt, st * 128 : (st + 1) * 128], pl[:, :])

        def load_w_hilo(w_dram, scale=None):
            """Returns (Wh, Wl) bf16 tiles [128, 8, DK]."""
            wt = wpool.tile([128, D // 128, DK], FP32, tag="wt")
            nc.gpsimd.dma_start(out=wt[:, :, :], in_=w_dram.rearrange("(t p) k -> p t k", p=128))
            absorb(wt[:, 0, 0:1])
            if scale is not None:
                nc.scalar.mul(wt[:, :, :], wt[:, :, :], scale)
            wh = wpool.tile([128, D // 128, DK], BF16, tag="wh")
            nc.scalar.copy(wh[:, :, :], wt[:, :, :])
            nc.vector.tensor_sub(wt[:, :, :], wt[:, :, :], wh[:, :, :])
            wl = wpool.tile([128, D // 128, DK], BF16, tag="wl")
            nc.scalar.copy(wl[:, :, :], wt[:, :, :])
            return wh, wl

        def proj_hilo(wh, wl, bT, outT):
            """outT [128 dk, SL] fp32 = (x @ W)^T + b, hi/lo 3-term."""
            for g in range(SL // 512):
                ps = psS.tile([128, 512], FP32, tag="ps")
                n = 3 * (D // 128)
                i = 0
                for dt in range(D // 128):
                    for a_, b_ in ((xhT, wh), (xhT, wl), (xlT, wh)):
                        nc.tensor.matmul(
                            ps[:, :],
                            lhsT=b_[:, dt, :],
                            rhs=a_[:, dt, g * 512 : (g + 1) * 512],
                            start=(i == 0),
                            stop=(i == n - 1),
                        )
                        i += 1
                nc.scalar.activation(outT[:, g * 512 : (g + 1) * 512], ps[:, :], Act.Identity, bias=bT[:, :])

        # K first so its collectives start early
        whk, wlk = load_w_hilo(w_k)
        ktl = big.tile([128, SL], FP32, tag="ktl")
        proj_hilo(whk, wlk, bkT, ktl)
        ktlr = big.tile([128, SL], FP32R, tag="ktlr")
        H = SL // 2
        nc.vector.tensor_copy(ktlr[:, 0:H], ktl[:, 0:H])
        nc.gpsimd.dma_start(out=ktl1_d[:, :], in_=ktlr[:, 0:H].bitcast(FP32))
        nc.gpsimd.collective_compute(
            "AllGather", Alu.bypass, replica_groups=groups, ins=[ktl1_d[:, :]], outs=[ktg1_d[:, :]]
        )
        nc.vector.tensor_copy(ktlr[:, H:SL], ktl[:, H:SL])
        nc.gpsimd.dma_start(out=ktl2_d[:, :], in_=ktlr[:, H:SL].bitcast(FP32))
        nc.gpsimd.collective_compute(
            "AllGather", Alu.bypass, replica_groups=groups, ins=[ktl2_d[:, :]], outs=[ktg2_d[:, :]]
        )

        # V next (bf16 natural layout)
        whv, wlv = load_w_hilo(w_v)
        vtl = big.tile([128, SL], FP32, tag="vtl")
        proj_hilo(whv, wlv, bvT, vtl)
        vtlb = big.tile([128, SL], BF16, tag="vtlb")
        nc.scalar.copy(vtlb[:, :], vtl[:, :])
        vnl = big.tile([128, SL // 128, DK], BF16, tag="vnl")
        for st in range(SL // 128):
            pt = psS.tile([128, 128], BF16, tag="ps")
            nc.tensor.transpose(pt[:, :], vtlb[:, st * 128 : (st + 1) * 128], identb[:, :])
            nc.vector.tensor_copy(vnl[:, st, :], pt[:, :])
        nc.gpsimd.dma_start(out=vnl_d.rearrange("(t p) k -> p t k", p=128), in_=vnl[:, :, :])
        nc.gpsimd.collective_compute(
            "AllGather", Alu.bypass, replica_groups=groups, ins=[vnl_d[:, :]], outs=[vng_d[:, :]]
        )

        # Q local (pre-scaled by 1/sqrt(dk))
        whq, wlq = load_w_hilo(w_q, scale=SCALE)
        qT = big.tile([128, SL], FP32, tag="qT")
        proj_hilo(whq, wlq, bqTs, qT)
        qr = big.tile([128, SL], FP32R, tag="qr")
        nc.vector.tensor_copy(qr[:, :], qT[:, :])
        proj_ctx.close()

        # ---- gathered K^T fp32r (two seq halves), V natural bf16 ----
        ktF3 = big.tile([128, NC, SL], FP32R, tag="ktF")
        nc.gpsimd.dma_start(
            out=ktF3[:, :, 0 : SL // 2],
            in_=ktg1_d.rearrange("(c p) s -> p c s", p=128).bitcast(FP32R),
        )
        absorb(ktF3[:, 0, 0:1])
        nc.gpsimd.dma_start(
            out=ktF3[:, :, SL // 2 : SL],
            in_=ktg2_d.rearrange("(c p) s -> p c s", p=128).bitcast(FP32R),
        )
        absorb(ktF3[:, 0, SL // 2 : SL // 2 + 1])
        ktF = ktF3.rearrange("p c s -> p (c s)")
        vnF = big.tile([128, S // 128, DK], BF16, tag="vnF")
        nc.gpsimd.dma_start(out=vnF[:, :, :], in_=vng_d.rearrange("(t p) k -> p t k", p=128))
        absorb(vnF[:, 0, 0:1])

        # ================= attention phase =================
        NKT = S // 128  # 64 k-tiles of 128

        def pass1_step(g, kt, macc):
            """One bf16-rate fp32r scores tile in [k, q] + DVE running max."""
            qs = slice(g * NQ, (g + 1) * NQ)
            ps = psP.tile([128, NQ], FP32, tag="p1")
            nc.tensor.matmul(
                ps[:, :], lhsT=ktF[:, kt * 128 : (kt + 1) * 128], rhs=qr[:, qs],
                start=True, stop=True,
            )
            if kt == 0:
                nc.vector.tensor_copy(macc[:, :], ps[:, :])
            else:
                nc.vector.tensor_max(macc[:, :], macc[:, :], ps[:, :])

        def pass1_reduce(macc):
            """macc [128, NQ] -> -m as a bf16 [1, NQ] row."""
            nmrow = rows.tile([1, NQ], FP32, tag="nmrow")
            for qt in range(NQ // 128):
                ptr = psS.tile([128, 128], FP32, tag="ps")
                nc.tensor.transpose(ptr[:, :], macc[:, qt * 128 : (qt + 1) * 128], ident[:, :])
                nmq = stat.tile([128, 1], FP32, tag="nmq")
                nc.vector.reduce_max(nmq[:, :], ptr[:, :], axis=mybir.AxisListType.X, negate=True)
                prm = psS.tile([128, 128], FP32, tag="ps")
                nc.tensor.transpose(prm[0:1, :], nmq[:, :], ident[:, :])
                nc.scalar.copy(nmrow[0:1, qt * 128 : (qt + 1) * 128], prm[0:1, :])
            nmrow_b = rows.tile([1, NQ], BF16, tag="nmrow_b")
            nc.scalar.copy(nmrow_b[0:1, :], nmrow[0:1, :])
            return nmrow_b

        ones_col = const.tile([128, 1], BF16, tag="ones_col")
        nc.gpsimd.memset(ones_col[:, :], 1.0)

        def score_exp_step(g, kt, nmrow_b):
            """Seed -m, fp32r scores, exp -> returns the P^T tile."""
            qs = slice(g * NQ, (g + 1) * NQ)
            ps = psS.tile([128, NQ], FP32, tag="ps")
            nc.tensor.matmul(
                ps[:, :], lhsT=ones_rb[0:1, :], rhs=nmrow_b[0:1, :],
                start=True, stop=False, skip_group_check=True,
            )
            nc.tensor.matmul(
                ps[:, :], lhsT=ktF[:, kt * 128 : (kt + 1) * 128], rhs=qr[:, qs],
                start=False, stop=True, skip_group_check=True,
            )
            ptile = pbufp.tile([128, NQ], BF16, tag="ptile")
            nc.scalar.activation(ptile[:, :], ps[:, :], Act.Exp)
            return ptile

        def pv_step(kt, ptile, psO, psL):
            """PV and l accumulate for a finished P^T tile."""
            nc.tensor.matmul(
                psO[:, :], lhsT=vnF[:, kt, :], rhs=ptile[:, :],
                start=(kt == 0), stop=(kt == NKT - 1), skip_group_check=True,
            )
            nc.tensor.matmul(
                psL[0:1, :], lhsT=ones_col[:, :], rhs=ptile[:, :],
                start=(kt == 0), stop=(kt == NKT - 1), skip_group_check=True,
            )

        def tail(g, psO, psL):
            """1/l broadcast over partitions, normalize, transpose out, DMA."""
            rrow = rows.tile([1, NQ], FP32, tag="rrow")
            nc.vector.reciprocal(rrow[0:1, :], psL[0:1, :])
            pb = psS.tile([128, NQ], FP32, tag="ps")
            nc.tensor.matmul(pb[:, :], lhsT=ones_rf[0:1, :], rhs=rrow[0:1, :], start=True, stop=True)
            rbc = stat.tile([128, NQ], FP32, tag="rbc")
            nc.vector.tensor_copy(rbc[:, :], pb[:, :])
            otn = stat.tile([128, NQ], FP32, tag="otn")
            nc.vector.tensor_mul(otn[:, :], psO[:, :], rbc[:, :])
            for qt in range(NQ // 128):
                po = psS.tile([128, 128], FP32, tag="ps")
                nc.tensor.transpose(po[:, :], otn[:, qt * 128 : (qt + 1) * 128], ident[:, :])
                ot = outp.tile([128, 128], FP32)
                nc.scalar.copy(ot[:, :], po[:, :])
                q0 = g * NQ + qt * 128
                nc.gpsimd.dma_start(out=out_sh[q0 : q0 + 128, :], in_=ot[:, :])

        NG = SL // NQ
        P1ORD = [kt for kt in range(NKT) if kt % 8 < 4] + [kt for kt in range(NKT) if kt % 8 >= 4]
        maccs = []
        for g in range(NG):
            m_g = stat.tile([128, NQ], FP32, tag=f"macc{g}", bufs=1, name=f"macc{g}")
            maccs.append(m_g)
        # pass1(0) standalone: gathered first-half k-tiles first
        warm_burst(12, lhs=ktF3[:, 0, 0:64].bitcast(BF16))
        for kt in P1ORD:
            pass1_step(0, kt, maccs[0])
        # re-warm after the DVE-paced max pass before pass2 starts
        warm_burst(12, lhs=ktF3[:, 0, 0:64].bitcast(BF16))
        nmb = pass1_reduce(maccs[0])
        B = 4  # k-tiles per emission batch (= ps PSUM slots)
        pending = None
        for g in range(NG):
            psO = psacc.tile([128, NQ], FP32, tag="psO")
            psL = psacc.tile([1, NQ], FP32, tag="psL")
            prev = []
            nbat = (NKT + B - 1) // B
            for b in range(nbat + 1):
                kts = range(b * B, min((b + 1) * B, NKT))
                cur = [(kt, score_exp_step(g, kt, nmb)) for kt in kts]
                if b == 1 and pending is not None:
                    pending()
                    pending = None
                for kt in kts:
                    if g + 1 < NG:
                        pass1_step(g + 1, kt, maccs[g + 1])
                for kt, pt in prev:
                    pv_step(kt, pt, psO, psL)
                prev = cur
            if g + 1 < NG:
                nmb = pass1_reduce(maccs[g + 1])
            pending = (lambda g=g, psO=psO, psL=psL: tail(g, psO, psL))
        pending()

    split_multi_waits(nc)
    return nc


def split_multi_waits(nc):
    """Hoist all-but-one sync wait off engine/DMA instructions into
    standalone EventSemaphore instructions (this walrus build has a single
    wait-command slot per non-sequencer instruction)."""
    import bass_rust

    exempt = {"InstEventSemaphore"}
    n_split = 0
    for f in nc.m.functions:
        for bb in f.blocks:
            out = []
            changed = False
            for ins in bb.instructions:
                si = ins.sync_info
                if (
                    si is not None
                    and len(si.on_wait) > 1
                    and type(ins).__name__ not in exempt
                    and ins.engine is not None
                ):
                    for j, w in enumerate(si.on_wait[:-1]):
                        ev = mybir.InstEventSemaphore(
                            name=f"{ins.name}-wsplit{j}", ins=[], outs=[]
                        )
                        ev.engine = ins.engine
                        ev.sync_info = bass_rust.SyncInfo(on_wait=[w], on_update=[])
                        out.append(ev)
                        n_split += 1
                    ins.sync_info = bass_rust.SyncInfo(
                        on_wait=[si.on_wait[-1]], on_update=list(si.on_update)
                    )
                    changed = True
                out.append(ins)
            if changed:
                bb.instructions = out
    return n_split


_PROGRAM = None


def _get_program():
    global _PROGRAM
    if _PROGRAM is None:
        _PROGRAM = build_program()
    return _PROGRAM


def kernel(x, W_Q, b_Q, W_K, b_K, W_V, b_V):
    x = np.ascontiguousarray(np.asarray(x, dtype=np.float32))
    args = {
        "W_Q": np.ascontiguousarray(np.asarray(W_Q, dtype=np.float32)),
        "b_Q": np.ascontiguousarray(np.asarray(b_Q, dtype=np.float32)),
        "W_K": np.ascontiguousarray(np.asarray(W_K, dtype=np.float32)),
        "b_K": np.ascontiguousarray(np.asarray(b_K, dtype=np.float32)),
        "W_V": np.ascontiguousarray(np.asarray(W_V, dtype=np.float32)),
        "b_V": np.ascontiguousarray(np.asarray(b_V, dtype=np.float32)),
    }
    nc = _get_program()
    in_maps = [dict(args, x_sh=x[c * SL : (c + 1) * SL]) for c in range(NC)]
    res = run_bass_kernel_spmd(nc, in_maps, list(range(NC)))
    return np.concatenate([res.results[c]["out_sh"] for c in range(NC)], axis=0)
